# revision 14
# baseline (speedup 1.0000x reference)
"""3D Haar DWT low-pass (DWT3DTiny) Trainium2 kernel.

The reference applies the Haar rec_lo filter [s, s] (s = sqrt(2)/2) with
stride-2 downsampling along t, h, w for every channel.  That is exactly a
2x2x2 box sum scaled by s^3 = 2**-1.5:

    out[ts, hs, ws, c] = 2**-1.5 * sum_{dt,dh,dw in {0,1}} x[2ts+dt, 2hs+dh, 2ws+dw, c]

Sharding: along t (pure data-parallel, t-pairs never cross a core
boundary since 32 / 8 = 4 rows per core), contiguous host-side slices.

v6 design, from NTFF packet/instruction analysis:
  * The 16 SDMA engines are ~97% occupied in-span; per-packet
    throughput grows with descriptor size (2 KiB -> 24.4, 8 KiB ->
    26.0, 32 KiB -> 26.6 GB/s/engine), so bulk tiles keep partition p
    holding h rows (2p, 2p+1) full width = one 32 KiB contiguous
    descriptor per partition.
  * The run's end is bounded by the serial DVE chain over the
    last-landing data: b-data that completes a t-pair carries
    ~2.6 us/MiB of DVE (hb, t, w) vs the ~2.4 us/MiB DMA stream, so
    any b-heavy stream tail falls behind.  The final chunk is
    therefore loaded as interleaved (a, b) w-slice pairs
    [128,128,128,64,32,16,16] wi: a whole slice costs ~2.1 us/MiB
    (ha+hb+t+w amortized), DVE keeps pace, and the drain after the
    last 16-wi slice is just its own tiny chain.
  * The chunk before the sliced one has its b split into two
    half-width tiles (8 KiB descriptors) to halve its landing coda.
  * The last two slices fold the 2**-1.5 scale into the adds
    (tensor_scalar 2x pre-scale of ha + scalar_tensor_tensor t-add)
    so no ACT mul sits on the final store path; other outputs get one
    ACT mul per chunk/slice off the critical path.
  * Loads on the SP HWDGE ring, stores on the ACT ring.
  * Dead const-tile memsets stripped from the init preamble (~9 us of
    GpSimd startup the all-engine barrier otherwise waits on).
Rejected experimentally: w-adds on GpSimd/Pool (2.3x slower, sem
latency lands on the store path: 97.2 us), graduated b-only tail
pieces (92.0-93.4 us: c2 coda + pieces stack up), uniform small tail
pieces (92.2), SWDGE loads, 4 MiB loads with bufs=2 (v1 notes).
"""

import numpy as np

import concourse.bacc as bacc
import concourse.mybir as mybir
from concourse.bass_utils import run_bass_kernel_spmd
from concourse.tile import TileContext

N_CORES = 8
T, H, W, C = 32, 512, 512, 8
TS = T // N_CORES  # t rows per core
SCALE = float(2.0 ** -1.5)
WC = W * C  # 4096 f32 per h row
HWC = WC // 2
SLICE_WI = [96, 96, 96, 96, 64, 32, 16, 16]  # final chunk (a,b) slice pairs
N_RING = 4  # first slices share ring tags; the rest get dedicated tiles
FOLD_FROM = 6  # slices >= this index fold the scale (no ACT mul)

_CACHE: dict = {}


def _build_nc() -> bacc.Bacc:
    nc = bacc.Bacc("TRN2", target_bir_lowering=False)
    x = nc.dram_tensor("x", [TS, H, W, C], mybir.dt.float32, kind="ExternalInput")
    y = nc.dram_tensor(
        "y", [TS // 2, H // 2, W // 2, C], mybir.dt.float32, kind="ExternalOutput"
    )

    # h = gb*256 + p*2 + two; rows 2p, 2p+1 full-width are adjacent in HBM.
    xq = x.rearrange("t (gb p two) w c -> t gb p two (w c)", p=128, two=2)
    # output row g = gb*128 + p: 256 v * 8 c = 8 KiB contiguous per partition
    yq = y.rearrange("s (gb p) w c -> s gb p (w c)", p=128)

    add = mybir.AluOpType.add
    mult = mybir.AluOpType.mult

    chunks = [(tp, gb) for tp in range(TS // 2) for gb in range(H // 256)]
    LAST = len(chunks) - 1
    ttp, tgb = chunks[LAST]

    def wadd(src, wdst):
        # w-pair add (wi = v*2 + dw): src [128, n] -> wdst [128, n//2]
        hv = src.rearrange("p (v two c) -> p v two c", two=2, c=C)
        wv = wdst.rearrange("p (v c) -> p v c", c=C)
        nc.vector.tensor_add(out=wv[:], in0=hv[:, :, 0], in1=hv[:, :, 1])

    with TileContext(nc) as tc:
        with (
            tc.tile_pool(name="pin", bufs=2) as pin,
            tc.tile_pool(name="pw", bufs=2) as pw,
            tc.tile_pool(name="psl", bufs=2) as ps,
            tc.tile_pool(name="ptl", bufs=1) as pt,
        ):
            # --- bulk chunks ---------------------------------------------
            for ci, (tp, gb) in enumerate(chunks[:-1]):
                a = pin.tile([128, 2, WC], mybir.dt.float32, tag="a")
                nc.sync.dma_start(out=a[:], in_=xq[2 * tp, gb])
                nc.vector.tensor_add(out=a[:, 0], in0=a[:, 0], in1=a[:, 1])
                ws = pw.tile([128, WC // 2], mybir.dt.float32, tag="w")
                # full-width b: one 32 KiB descriptor per partition
                b = pin.tile([128, 2, WC], mybir.dt.float32, tag="b")
                nc.sync.dma_start(out=b[:], in_=xq[2 * tp + 1, gb])
                nc.vector.tensor_add(out=b[:, 0], in0=b[:, 0], in1=b[:, 1])
                nc.vector.tensor_add(out=b[:, 0], in0=b[:, 0], in1=a[:, 0])
                wadd(b[:, 0], ws[:])
                nc.scalar.mul(ws[:], ws[:], SCALE)
                nc.scalar.dma_start(out=yq[tp, gb], in_=ws[:])

            # --- final chunk: interleaved (a, b) w-slice pairs -----------
            w0 = 0
            for k, wi in enumerate(SLICE_WI):
                wc = wi * C
                if k < N_RING:
                    sa = ps.tile([128, 2, wc], mybir.dt.float32, tag="sa")
                    sb = ps.tile([128, 2, wc], mybir.dt.float32, tag="sb")
                else:
                    sa = pt.tile([128, 2, wc], mybir.dt.float32, tag=f"sa{k}")
                    sb = pt.tile([128, 2, wc], mybir.dt.float32, tag=f"sb{k}")
                wsl = w0 * C
                wsh = (w0 + wi) * C
                nc.sync.dma_start(out=sa[:], in_=xq[2 * ttp, tgb, :, :, wsl:wsh])
                nc.sync.dma_start(out=sb[:], in_=xq[2 * ttp + 1, tgb, :, :, wsl:wsh])
                nc.vector.tensor_add(out=sa[:, 0], in0=sa[:, 0], in1=sa[:, 1])
                fold = k >= FOLD_FROM
                if fold:
                    nc.vector.tensor_scalar_mul(sa[:, 0], sa[:, 0], SCALE)
                nc.vector.tensor_add(out=sb[:, 0], in0=sb[:, 0], in1=sb[:, 1])
                if fold:
                    # t-add with the scale folded: s*hb + (s*ha) -> final
                    nc.vector.scalar_tensor_tensor(
                        out=sb[:, 0], in0=sb[:, 0], scalar=SCALE,
                        in1=sa[:, 0], op0=mult, op1=add,
                    )
                else:
                    nc.vector.tensor_add(out=sb[:, 0], in0=sb[:, 0], in1=sa[:, 0])
                if k < N_RING:
                    # reuse the bulk w ring (bigger tiles; use a prefix view)
                    wfull = pw.tile([128, WC // 2], mybir.dt.float32, tag="w")
                    wt = wfull[:, : wc // 2]
                else:
                    wt = pt.tile([128, wc // 2], mybir.dt.float32, tag=f"sw{k}")
                wadd(sb[:, 0], wt[:])
                if not fold:
                    nc.scalar.mul(wt[:], wt[:], SCALE)
                nc.scalar.dma_start(
                    out=yq[ttp, tgb, :, (w0 // 2) * C : ((w0 + wi) // 2) * C],
                    in_=wt[:],
                )
                w0 += wi

    _strip_init_preamble(nc)
    if not nc.is_finalized():
        nc.finalize()  # Bacc.compile: event-sem split (1 wait/inst), reg alloc
    return nc


def _strip_init_preamble(nc) -> None:
    """Drop the four Bass.__init__ const-tile memsets from block 0.  Nothing
    in this kernel reads the const tiles, yet the initial all-engine barrier
    waits on the GpSimd engine executing them, which costs ~9 us of Q7
    startup on HW.  The drains and the all-engine barrier are kept."""
    b0 = nc.main_func.blocks[0]
    b0.instructions[:] = [
        ins for ins in b0.instructions if type(ins).__name__ != "InstMemset"
    ]


def kernel(x) -> np.ndarray:
    x = np.asarray(x, dtype=np.float32)
    assert x.shape == (T, H, W, C), x.shape

    if "nc" not in _CACHE:
        _CACHE["nc"] = _build_nc()
    nc = _CACHE["nc"]

    in_maps = [
        {"x": np.ascontiguousarray(x[i * TS : (i + 1) * TS])} for i in range(N_CORES)
    ]
    res = run_bass_kernel_spmd(nc, in_maps, core_ids=list(range(N_CORES)))
    return np.concatenate([r["y"] for r in res.results], axis=0)


# revision 16
# speedup vs baseline: 1.0287x; 1.0287x over previous
"""3D Haar DWT low-pass (DWT3DTiny) Trainium2 kernel.

The reference applies the Haar rec_lo filter [s, s] (s = sqrt(2)/2) with
stride-2 downsampling along t, h, w for every channel.  That is exactly a
2x2x2 box sum scaled by s^3 = 2**-1.5:

    out[ts, hs, ws, c] = 2**-1.5 * sum_{dt,dh,dw in {0,1}} x[2ts+dt, 2hs+dh, 2ws+dw, c]

Sharding: along t (pure data-parallel, t-pairs never cross a core
boundary since 32 / 8 = 4 rows per core), contiguous host-side slices.

v7 design, from NTFF packet/instruction analysis.  The 16 SDMA engines
are ~97% occupied in-span and per-packet throughput grows with
descriptor size (2 KiB -> 24.4, 8 KiB -> 26.0, 32 KiB -> 26.6
GB/s/engine), so the layout maximizes contiguous descriptor size:
  * chunk = (t-pair, 256-h-row block); partition p holds h rows
    (2p, 2p+1) full width -> one 32 KiB contiguous descriptor per
    partition per t-row tile; 4 chunks of 8 MiB;
  * per chunk: ha = a0+a1, hb = b0+b1, t = ha+hb, strided w-pair add
    (all DVE, in place), one ACT scale + one 8 KiB-descriptor store;
  * the final chunk loads a as two half-width tiles (16 KiB descs,
    so its reduction starts half a tile earlier) and b as graduated
    w-pieces [256,128,64,48,16] wi; each piece folds the scale into
    the t-add (scalar_tensor_tensor: s*hb + s*ha) and scales nothing
    afterwards, so ACT only issues the store and the post-last-load
    drain is the 16-wi chain only;
  * loads on the SP HWDGE ring, stores on the ACT ring (sharing one
    ring head-of-line blocks loads behind stores);
  * dead const-tile memsets stripped from the init preamble (~9 us of
    GpSimd startup the all-engine barrier otherwise waits on).
Rejected experimentally: w-adds on GpSimd/Pool (2.3x slower, sem
latency lands on the store path: 97.2 us), b-rows split into halves
everywhere (8 KiB descs on 12 MiB cost more than the halved coda
saved: 93.4 us), final chunk as interleaved (a,b) slice pairs (ring
coupling + small descs: 95.6 us), uniform small tail pieces (92.2 us),
SWDGE loads, 4 MiB loads with bufs=2 (v1 notes).
"""

import numpy as np

import concourse.bacc as bacc
import concourse.mybir as mybir
from concourse.bass_utils import run_bass_kernel_spmd
from concourse.tile import TileContext

N_CORES = 8
T, H, W, C = 32, 512, 512, 8
TS = T // N_CORES  # t rows per core
SCALE = float(2.0 ** -1.5)
TAIL_WI = [256, 128, 64, 48, 16]  # graduated pieces of the final chunk's b
WC = W * C  # 4096 f32 per h row
HWC = WC // 2

_CACHE: dict = {}


def _build_nc() -> bacc.Bacc:
    nc = bacc.Bacc("TRN2", target_bir_lowering=False)
    x = nc.dram_tensor("x", [TS, H, W, C], mybir.dt.float32, kind="ExternalInput")
    y = nc.dram_tensor(
        "y", [TS // 2, H // 2, W // 2, C], mybir.dt.float32, kind="ExternalOutput"
    )

    # h = gb*256 + p*2 + two; rows 2p, 2p+1 full-width are adjacent in HBM.
    xq = x.rearrange("t (gb p two) w c -> t gb p two (w c)", p=128, two=2)
    # output row g = gb*128 + p: 256 v * 8 c = 8 KiB contiguous per partition
    yq = y.rearrange("s (gb p) w c -> s gb p (w c)", p=128)

    add = mybir.AluOpType.add
    mult = mybir.AluOpType.mult

    chunks = [(tp, gb) for tp in range(TS // 2) for gb in range(H // 256)]
    LAST = len(chunks) - 1
    ttp, tgb = chunks[LAST]

    def wadd(src, wdst):
        # w-pair add (wi = v*2 + dw): src [128, n] -> wdst [128, n//2]
        hv = src.rearrange("p (v two c) -> p v two c", two=2, c=C)
        wv = wdst.rearrange("p (v c) -> p v c", c=C)
        nc.vector.tensor_add(out=wv[:], in0=hv[:, :, 0], in1=hv[:, :, 1])

    with TileContext(nc) as tc:
        with (
            tc.tile_pool(name="pin", bufs=2) as pin,
            tc.tile_pool(name="pw", bufs=3) as pw,
            tc.tile_pool(name="ptl", bufs=1) as pt,
        ):
            # --- bulk chunks ---------------------------------------------
            for tp, gb in chunks[:-1]:
                a = pin.tile([128, 2, WC], mybir.dt.float32, tag="a")
                b = pin.tile([128, 2, WC], mybir.dt.float32, tag="b")
                nc.sync.dma_start(out=a[:], in_=xq[2 * tp, gb])
                nc.sync.dma_start(out=b[:], in_=xq[2 * tp + 1, gb])
                nc.vector.tensor_add(out=a[:, 0], in0=a[:, 0], in1=a[:, 1])
                nc.vector.tensor_add(out=b[:, 0], in0=b[:, 0], in1=b[:, 1])
                nc.vector.tensor_add(out=a[:, 0], in0=a[:, 0], in1=b[:, 0])
                ws = pw.tile([128, WC // 2], mybir.dt.float32, tag="w")
                wadd(a[:, 0], ws[:])
                nc.scalar.mul(ws[:], ws[:], SCALE)
                nc.scalar.dma_start(out=yq[tp, gb], in_=ws[:])

            # --- final chunk ---------------------------------------------
            # a in two halves (16 KiB descs) so its reduction starts early;
            # pre-scale it so the pieces' t-add can fold the output scale.
            ta = pin.tile([128, 2, WC], mybir.dt.float32, tag="a")
            for j in range(2):
                nc.sync.dma_start(
                    out=ta[:, :, j * HWC : (j + 1) * HWC],
                    in_=xq[2 * ttp, tgb, :, :, j * HWC : (j + 1) * HWC],
                )
                nc.vector.tensor_add(
                    out=ta[:, 0, j * HWC : (j + 1) * HWC],
                    in0=ta[:, 0, j * HWC : (j + 1) * HWC],
                    in1=ta[:, 1, j * HWC : (j + 1) * HWC],
                )
                nc.vector.tensor_scalar_mul(
                    ta[:, 0, j * HWC : (j + 1) * HWC],
                    ta[:, 0, j * HWC : (j + 1) * HWC],
                    SCALE,
                )

            w0 = 0
            for k, wi in enumerate(TAIL_WI):
                wc = wi * C
                bp = pt.tile([128, 2, wc], mybir.dt.float32, tag=f"tb{k}")
                nc.sync.dma_start(
                    out=bp[:],
                    in_=xq[2 * ttp + 1, tgb, :, :, w0 * C : (w0 + wi) * C],
                )
                nc.vector.tensor_add(out=bp[:, 0], in0=bp[:, 0], in1=bp[:, 1])
                # t-add with the scale folded in: s*hb + (s*ha) -> final
                nc.vector.scalar_tensor_tensor(
                    out=bp[:, 0], in0=bp[:, 0], scalar=SCALE,
                    in1=ta[:, 0, w0 * C : (w0 + wi) * C], op0=mult, op1=add,
                )
                wt = pt.tile([128, wc // 2], mybir.dt.float32, tag=f"tw{k}")
                wadd(bp[:, 0], wt[:])
                nc.scalar.dma_start(
                    out=yq[ttp, tgb, :, (w0 // 2) * C : ((w0 + wi) // 2) * C],
                    in_=wt[:],
                )
                w0 += wi

    _strip_init_preamble(nc)
    if not nc.is_finalized():
        nc.finalize()  # Bacc.compile: event-sem split (1 wait/inst), reg alloc
    return nc


def _strip_init_preamble(nc) -> None:
    """Drop the four Bass.__init__ const-tile memsets from block 0.  Nothing
    in this kernel reads the const tiles, yet the initial all-engine barrier
    waits on the GpSimd engine executing them, which costs ~9 us of Q7
    startup on HW.  The drains and the all-engine barrier are kept."""
    b0 = nc.main_func.blocks[0]
    b0.instructions[:] = [
        ins for ins in b0.instructions if type(ins).__name__ != "InstMemset"
    ]


def kernel(x) -> np.ndarray:
    x = np.asarray(x, dtype=np.float32)
    assert x.shape == (T, H, W, C), x.shape

    if "nc" not in _CACHE:
        _CACHE["nc"] = _build_nc()
    nc = _CACHE["nc"]

    in_maps = [
        {"x": np.ascontiguousarray(x[i * TS : (i + 1) * TS])} for i in range(N_CORES)
    ]
    res = run_bass_kernel_spmd(nc, in_maps, core_ids=list(range(N_CORES)))
    return np.concatenate([r["y"] for r in res.results], axis=0)
